# revision 1
# baseline (speedup 1.0000x reference)
"""CRF loss kernel for Trainium2 (8 NeuronCores, data-parallel over batch).

Strategy
--------
Batch 2048 is sharded 8 ways (256 rows/core). The partition function
(forward algorithm over S=512 steps) runs on device in exp-space:

  A_s[j,b] = (sum_i A_{s-1}[i,b] * E[i,j]) * exp(em[b,s,j] - g_s)

with E = exp(transitions). Per-core layout packs the 256 batch rows
(padded to 258 = 3*86) as 3 chunks on partitions: A[32c+i, n] holds batch
row c*86+n, tag i -> a [96, 86] tile. One fused matmul per step with
weights lhsT [96, 99] = blockdiag(E,E,E) | per-chunk exp(end_trans)
columns, so psum rows 96:99 of step s give z_{s-1} = sum_i A_{s-1} * eend
(the masked-length readout) for free. One DVE multiply per step applies
the emission factor and stores the step's A (+ shifted z) into a 64-slot
SBUF ring; z rows are DMA'd out per 64-step block.

Stability: a host-computed per-step scalar g_s keeps A near 1; every 16
steps a transpose/renorm divides each column by its max and logs the
factor (lcout). The host reconstructs logZ_b = log(z_{L_b-1}) + offsets,
computes the gold-path score in numpy, and returns the mean loss.
"""
import os
import sys

import numpy as np

for _p in ("/opt/trn_rl_repo",):
    if _p not in sys.path and os.path.isdir(_p):
        sys.path.insert(0, _p)

B, S, T = 2048, 512, 32
NCORES = 8
BL = B // NCORES          # 256 batch rows per core
F = 86                    # batch columns per chunk
NC = 3                    # chunks per core
BLP = NC * F              # 258 padded rows
K = NC * T                # 96 contraction partitions
M = K + NC                # 99 matmul output rows (incl. z rows)
KREN = 32                 # renorm period
RING = 64                 # ring slots
NBLK = S // RING          # 8 z export blocks
NCKPT = S // KREN - 1     # 31 renorm checkpoints (s = 16..496)

_BASS_CACHE = {}


def _build_bass(strip=True):
    import concourse.bass as bass
    import concourse.mybir as mybir
    from concourse.alu_op_type import AluOpType
    from concourse.bass import _add_dep_helper as add_dep_helper
    from concourse.tile import TileContext

    dt = mybir.dt
    nc = bass.Bass()

    emt = nc.declare_dram_parameter("emt", [S // KREN, M, KREN * F], dt.float32, isOutput=False)
    a0 = nc.declare_dram_parameter("a0", [M, F], dt.float32, isOutput=False)
    lhst = nc.declare_dram_parameter("lhst", [K, M], dt.float32, isOutput=False)
    zout = nc.declare_dram_parameter("zout", [NBLK, NC, RING + 1, F], dt.float32, isOutput=True)
    lcout = nc.declare_dram_parameter("lcout", [K, NC * (NCKPT + 1)], dt.float32, isOutput=True)
    alast = nc.declare_dram_parameter("alast", [K, F], dt.float32, isOutput=True)

    with TileContext(nc) as tc:
        with (
            tc.tile_pool(name="const", bufs=1) as constp,
            tc.tile_pool(name="ringp", bufs=1) as ringp,
            tc.tile_pool(name="emts", bufs=3) as emtp,
            tc.tile_pool(name="work", bufs=3) as workp,
            tc.tile_pool(name="mmpsum", bufs=3, space="PSUM") as mmp,
        ):
            lh = constp.tile([K, M], dt.float32, name="lh")
            lcb = constp.tile([K, NC * (NCKPT + 1)], dt.float32, name="lcb")
            stg = constp.tile([K, K], dt.float32, name="stg")
            ring = ringp.tile([M, (RING + 1) * F], dt.float32, name="ring")

            # Stage matmul-consumed constants through DVE copies so every
            # matmul dependency is a single DVE semaphore (the fused
            # LDWEIGHTS+MATMUL struct has a tiny sync-wait budget).
            lhs_s = constp.tile([K, M], dt.float32, name="lhs_s")
            a0_s = constp.tile([M, F], dt.float32, name="a0_s")
            nc.sync.dma_start(out=lhs_s[:, :], in_=lhst[:, :])
            nc.sync.dma_start(out=a0_s[:, :], in_=a0[:, :])
            nc.vector.tensor_copy(lh[:, :], lhs_s[:, :])
            nc.vector.tensor_copy(ring[0:M, 0:F], a0_s[:, :])
            nc.vector.memset(ring[K:M, RING * F:(RING + 1) * F], 0.0)
            nc.vector.memset(lcb[:, NC * NCKPT:], 0.0)
            nc.vector.memset(stg[:, :], 1.0)
            dum = constp.tile([1, S // KREN], dt.float32, name="dum")

            def slot(u):
                return (u % RING) * F

            et = None
            for s16 in range(S // KREN):
                et = emtp.tile([M, KREN * F], dt.float32, tag="et")
                nc.sync.dma_start(out=et[:, :], in_=emt[s16])
                # Sponges: each engine instruction struct holds a single
                # sem wait, so route DMA waits through cheap DVE ops with
                # no data-dep back into the scan (ordering via add_dep).
                nc.vector.tensor_copy(dum[0:1, s16:s16 + 1], et[0:1, 0:1])
                if s16 % (RING // KREN) == 0 and s16 > 0:
                    zoff = RING * F + s16 // (RING // KREN)
                    nc.vector.memset(ring[K:K + 1, zoff:zoff + 1], 0.0)
                for ss in range(KREN):
                    s = s16 * KREN + ss
                    if s == 0:
                        continue
                    po = slot(s - 1)
                    rhs = ring[0:K, po:po + F]
                    if ss == 0:
                        # All-DVE renorm (single-proc deps -> single sem
                        # wait). 32x32 block transpose keeps tags within
                        # their chunk blocks: pt[32c+a, 32w+b] =
                        # A[32c+b, 32w+a], so the per-(chunk, batch-col)
                        # max is a free-dim reduce and the rescale is a
                        # per-partition tensor_scalar per col-block.
                        kk = s // KREN  # 1..NCKPT
                        nc.vector.tensor_copy(stg[:, 0:F], ring[0:K, po:po + F])
                        pt = workp.tile([K, K], dt.float32, tag="pt")
                        nc.vector.transpose(pt[:, :], stg[:, :])
                        mx = workp.tile([K, NC], dt.float32, tag="mx")
                        nc.vector.tensor_reduce(
                            mx[:, :],
                            pt[:, :].rearrange("p (w b) -> p w b", b=T),
                            mybir.AxisListType.X, AluOpType.max,
                        )
                        rmx = workp.tile([K, NC], dt.float32, tag="rmx")
                        nc.vector.reciprocal(rmx[:, :], mx[:, :])
                        # lcb holds 1/max; host applies -log.
                        nc.vector.tensor_copy(lcb[:, NC * (kk - 1):NC * kk], rmx[:, :])
                        atn = workp.tile([K, K], dt.float32, tag="atn")
                        for w in range(NC):
                            nc.vector.tensor_scalar(
                                atn[:, T * w:T * (w + 1)], pt[:, T * w:T * (w + 1)],
                                rmx[:, w:w + 1], None, AluOpType.mult,
                            )
                        rr = workp.tile([K, K], dt.float32, tag="rr")
                        nc.vector.transpose(rr[:, :], atn[:, :])
                        rhs = rr[:, 0:F]
                    ps = mmp.tile([M, F], dt.float32, tag="ps")
                    nc.tensor.matmul(ps[:, :], lh[:, :], rhs, start=True, stop=True)
                    so = slot(s)
                    nc.vector.tensor_tensor(
                        ring[0:M, so:so + F], ps[:, :],
                        et[:, ss * F:(ss + 1) * F], AluOpType.mult,
                    )
                    if s % RING == RING - 1:
                        blk = s // RING
                        nc.sync.dma_start(
                            out=zout[blk],
                            in_=ring[K:M, :].rearrange("p (u n) -> p u n", n=F),
                        )
            nc.sync.dma_start(out=alast[:, :], in_=ring[0:K, slot(S - 1):slot(S - 1) + F])
            nc.sync.dma_start(out=lcout[:, :], in_=lcb[:, :])

    if strip:
        _strip_redundant_dma_waits(nc, mybir)
    return nc


def _strip_redundant_dma_waits(nc, mybir):
    """The walrus build here accepts one sem wait per instruction. On
    DMACopy the extra waits are DMAHW-queue sems for DMA-vs-DMA WAW/WAR
    on buffers that are also serialized through the engine-sem chain
    (slot readers complete before the old DMA's region is rewritten), so
    they are transitively redundant: drop them, keep engine sems.
    """
    import json

    import copy
    import json

    j = json.loads(nc.to_json_str())
    uid = [0]

    for fn in j["functions"]:
        for blk in fn["blocks"]:
            out = []
            for inst in blk["instructions"]:
                si = inst.get("sync_info")
                ow = (si.get("on_wait") or []) if si else []
                if si and len(ow) > 1 and inst.get("opcode") == "DMACopy":
                    ow = [w for w in ow if not w["ant_name"].startswith("DMAHW")]
                    assert len(ow) <= 1, f"{inst['name']}: {ow}"
                    si["on_wait"] = ow
                if si and len(ow) > 1:
                    # split a multi-wait instruction (e.g. the tail Drain)
                    # into single-wait no-op Drains ahead of it
                    assert inst.get("opcode") == "Drain", (inst["name"], ow)
                    for w in ow[:-1]:
                        uid[0] += 1
                        out.append({
                            "name": f"{inst['name']}-w{uid[0]}",
                            "opcode": "Drain",
                            "engine": inst["engine"],
                            "debug": inst.get("debug", 0),
                            "ins": [],
                            "outs": [],
                            "is_reset_sema": False,
                            "sync_info": {"on_wait": [copy.deepcopy(w)],
                                          "on_update": []},
                        })
                    si["on_wait"] = [ow[-1]]
                out.append(inst)
            blk["instructions"] = out

    nc.m = mybir.module_from_json_bytes(json.dumps(j).encode())


def _host_score(emissions, tags, mask, transitions, start_trans, end_trans, lengths):
    b_idx = np.arange(B)
    em_tag = np.take_along_axis(emissions, tags[:, :, None], axis=2)[:, :, 0]
    trans_g = transitions[tags[:, :-1], tags[:, 1:]]
    return (start_trans[tags[:, 0]].astype(np.float64)
            + em_tag[:, 0]
            + ((trans_g + em_tag[:, 1:]) * mask[:, 1:]).sum(1, dtype=np.float64)
            + end_trans[tags[b_idx, lengths - 1]])


def kernel(emissions, tags, mask, transitions, start_trans, end_trans):
    from concourse.bass_utils import run_bass_kernel_spmd

    emissions = np.ascontiguousarray(np.asarray(emissions, np.float32))
    transitions = np.asarray(transitions, np.float32)
    start_trans = np.asarray(start_trans, np.float32)
    end_trans = np.asarray(end_trans, np.float32)
    tags = np.asarray(tags).astype(np.int64)
    mask = np.asarray(mask).astype(bool)

    lengths = mask.sum(1).astype(np.int64)
    score = _host_score(emissions, tags, mask, transitions, start_trans, end_trans, lengths)

    E = np.exp(transitions)
    eend = np.exp(end_trans)
    gE = float(np.log(np.exp(transitions.astype(np.float64)).sum(0)).mean())

    lhsT = np.zeros((K, M), np.float32)
    for c in range(NC):
        lhsT[T * c:T * (c + 1), T * c:T * (c + 1)] = E
        lhsT[T * c:T * (c + 1), K + c] = eend

    in_maps = []
    host = []  # per-core (cumg, z0)
    for core in range(NCORES):
        em = emissions[core * BL:(core + 1) * BL]
        em_p = np.zeros((BLP, S, T), np.float32)
        em_p[:BL] = em
        g = em[:, :, :].max(2).mean(0).astype(np.float64) + gE
        g[0] = (em[:, 0] + start_trans[None, :]).max(1).mean(0)
        cumg = np.cumsum(g)

        emt = np.exp(em_p - g[None, :, None].astype(np.float32))
        emt_dev = np.ones((S, M, F), np.float32)
        emt_dev[:, :K, :] = (
            emt.reshape(NC, F, S, T).transpose(2, 0, 3, 1).reshape(S, K, F))
        emt_dev = np.ascontiguousarray(
            emt_dev.reshape(S // KREN, KREN, M, F).transpose(0, 2, 1, 3)
            .reshape(S // KREN, M, KREN * F))

        A0 = np.exp(start_trans[None, :] + em_p[:, 0, :] - np.float32(g[0]))
        z0 = (A0.reshape(NC, F, T) * eend[None, None, :]).sum(2)  # [NC, F]
        A0d = np.empty((M, F), np.float32)
        A0d[:K] = A0.reshape(NC, F, T).transpose(0, 2, 1).reshape(K, F)
        A0d[K:] = z0

        in_maps.append({"emt": emt_dev, "a0": A0d, "lhst": lhsT})
        host.append((cumg, z0))

    nc = _BASS_CACHE.get("nc")
    if nc is None:
        nc = _build_bass()
        _BASS_CACHE["nc"] = nc

    global _LAST_IN_MAPS
    _LAST_IN_MAPS = in_maps
    res = run_bass_kernel_spmd(nc, in_maps, list(range(NCORES))).results

    logZ = np.zeros(B, np.float64)
    for core in range(NCORES):
        cumg, z0 = host[core]
        zo = np.asarray(res[core]["zout"], np.float64)      # [NBLK, NC, RING+1, F]
        # lcout[32c + n%T, NC*k + n//T] = 1/colmax of batch col n, chunk c,
        # checkpoint k (device block-transpose layout); lc = -log -> [c, n, k]
        lcraw = np.asarray(res[core]["lcout"], np.float64).reshape(NC, T, NCKPT + 1, NC)
        nn = np.arange(F)
        lc = -np.log(lcraw[np.arange(NC)[:, None, None],
                           (nn % T)[None, :, None],
                           np.arange(NCKPT + 1)[None, None, :],
                           (nn // T)[None, :, None]])        # [NC, F, NCKPT+1]
        lc_cum = np.cumsum(lc, axis=2)                       # [NC, F, NCKPT+1]
        al = np.asarray(res[core]["alast"], np.float64)      # [K, F]
        zlast = (al.reshape(NC, T, F) * eend[None, :, None].astype(np.float64)).sum(1)

        r = np.arange(BL)
        c, n = r // F, r % F
        t = lengths[core * BL:(core + 1) * BL] - 1
        u = t + 1
        zv = np.where(t == 0, z0[c, n],
                      np.where(t == S - 1, zlast[c, n],
                               zo[np.minimum(u // RING, NBLK - 1), c, u % RING, n]))
        nk = np.minimum(u // KREN, NCKPT)
        off = cumg[t] + np.where(nk > 0, lc_cum[c, n, np.maximum(nk - 1, 0)], 0.0)
        logZ[core * BL:(core + 1) * BL] = np.log(zv) + off

    loss = (logZ - score).mean()
    return np.array(loss, dtype=np.float32)



# revision 5
# speedup vs baseline: 1.6232x; 1.6232x over previous
"""CRF loss kernel for Trainium2 (8 NeuronCores, data-parallel over batch).

Strategy
--------
Batch 2048 is sharded 8 ways (256 rows/core). The partition function
(forward algorithm over S=512 steps) runs on device in exp-space:

  A_s[j,b] = (sum_i A_{s-1}[i,b] * E[i,j]) * exp(em[b,s,j] - g_s)

with E = exp(transitions). Per-core layout packs the 256 batch rows
(padded to 258 = 3*86) as 3 chunks on partitions: A[32c+i, n] holds batch
row c*86+n, tag i -> a [96, 86] tile. One bf16 matmul per step with
weights lhsT [96, 99] = blockdiag(E,E,E) | per-chunk exp(end_trans)
columns, so psum rows 96:99 of step s give z_{s-1} = sum_i A_{s-1} * eend
(the masked-length readout) for free. The weights are constant, so all
matmuls after the first run with ldweights=False (PE array reuse). One
DVE multiply per step applies the emission factor and stores the step's
A+z into a 64-slot bf16 SBUF ring; z rows are DMA'd out per 64-step block.

Stability: a host-computed per-step scalar g_s keeps A near 1; every 32
steps the z-row of the current slot (a natural per-column magnitude
proxy) is reciprocal'd, broadcast across partitions by a tiny
outer-product matmul (indT weights at PE rows 96:99 would clobber the
main weights' rows 0:32, so the next main matmul reloads), and folded
into the emission tile two steps ahead. The host reconstructs
logZ_b = log(z_row[L_b]) + cumg + sum of log(z_row[32k]) scale terms,
all read from zout; the gold-path score is computed in numpy.
"""
import os
import sys

import numpy as np

for _p in ("/opt/trn_rl_repo",):
    if _p not in sys.path and os.path.isdir(_p):
        sys.path.insert(0, _p)

B, S, T = 2048, 512, 32
NCORES = 8
BL = B // NCORES          # 256 batch rows per core
F = 86                    # batch columns per chunk
NC = 3                    # chunks per core
BLP = NC * F              # 258 padded rows
K = NC * T                # 96 contraction partitions
M = K + NC                # 99 matmul output rows (incl. z rows)
KREN = 32                 # em DMA block / renorm period
RING = 64                 # ring slots
NBLK = S // RING          # 8 z export blocks
CKPTS = list(range(KREN, S, KREN))          # 32..480, applied at p+2

_BASS_CACHE = {}


def _build_bass(strip=True):
    import concourse.bass as bass
    import concourse.mybir as mybir
    from concourse.alu_op_type import AluOpType
    from concourse.tile import TileContext

    dt = mybir.dt
    nc = bass.Bass()

    emt = nc.declare_dram_parameter("emt", [S // KREN, M, KREN * F], dt.bfloat16, isOutput=False)
    a0 = nc.declare_dram_parameter("a0", [M, F], dt.bfloat16, isOutput=False)
    lhst = nc.declare_dram_parameter("lhst", [K, M], dt.bfloat16, isOutput=False)
    indt = nc.declare_dram_parameter("indt", [NC, M], dt.bfloat16, isOutput=False)
    zout = nc.declare_dram_parameter("zout", [NBLK, NC, RING * F], dt.bfloat16, isOutput=True)
    alast = nc.declare_dram_parameter("alast", [K, F], dt.bfloat16, isOutput=True)

    ckpt_set = set(CKPTS)

    with TileContext(nc) as tc:
        with (
            tc.tile_pool(name="const", bufs=1) as constp,
            tc.tile_pool(name="ringp", bufs=1) as ringp,
            tc.tile_pool(name="emts", bufs=3) as emtp,
            tc.tile_pool(name="rcp", bufs=2) as rcp,
            tc.tile_pool(name="emqp", bufs=2) as emqp,
            tc.tile_pool(name="mmpsum", bufs=3, space="PSUM") as mmp,
            tc.tile_pool(name="bcpsum", bufs=2, space="PSUM") as bcp,
        ):
            lh = constp.tile([K, M], dt.bfloat16, name="lh")
            ind = constp.tile([NC, M], dt.bfloat16, name="ind")
            ring = ringp.tile([M, RING * F], dt.bfloat16, name="ring")

            # Stage matmul-consumed constants through DVE copies so every
            # matmul dependency is a single DVE semaphore (the fused
            # LDWEIGHTS+MATMUL struct has a tiny sync-wait budget).
            lhs_s = constp.tile([K, M], dt.bfloat16, name="lhs_s")
            ind_s = constp.tile([NC, M], dt.bfloat16, name="ind_s")
            a0_s = constp.tile([M, F], dt.bfloat16, name="a0_s")
            nc.sync.dma_start(out=lhs_s[:, :], in_=lhst[:, :])
            nc.sync.dma_start(out=ind_s[:, :], in_=indt[:, :])
            nc.sync.dma_start(out=a0_s[:, :], in_=a0[:, :])
            nc.vector.tensor_copy(lh[:, :], lhs_s[:, :])
            nc.vector.tensor_copy(ind[:, :], ind_s[:, :])
            nc.vector.tensor_copy(ring[0:M, 0:F], a0_s[:, :])
            dum = constp.tile([1, S // KREN], dt.float32, name="dum")

            def slot(u):
                return (u % RING) * F

            emq = None          # renorm-scaled emission tile for step p+2
            need_ldw = True     # PE weight rows 0:96 need (re)loading
            for s16 in range(S // KREN):
                et = emtp.tile([M, KREN * F], dt.bfloat16, tag="et")
                nc.sync.dma_start(out=et[:, :], in_=emt[s16])
                # Sponges: each engine instruction struct holds a single
                # sem wait, so route DMA waits through cheap DVE ops with
                # no data-dep back into the scan (ordering via add_dep).
                nc.vector.tensor_copy(dum[0:1, s16:s16 + 1], et[0:1, 0:1])
                if s16 % (RING // KREN) == 0 and s16 > 0:
                    # WAR sponge: zout DMA of the previous 64-block must
                    # finish before this block's evacs rewrite z rows.
                    nc.vector.memset(ring[K:K + 1, 0:1], 0.0)
                for ss in range(KREN):
                    s = s16 * KREN + ss
                    if s == 0:
                        continue
                    po = slot(s - 1)
                    ps = mmp.tile([M, F], dt.float32, tag="ps")
                    mm = nc.tensor.matmul(ps[:, :], lh[:, :], ring[0:K, po:po + F],
                                          start=True, stop=True)
                    if need_ldw:
                        need_ldw = False
                    else:
                        mm.ins.ldweights = False
                    so = slot(s)
                    if s % KREN == 2 and (s - 2) in ckpt_set:
                        em_src = emq[:, :]
                    else:
                        em_src = et[:, ss * F:(ss + 1) * F]
                    nc.vector.tensor_tensor(
                        ring[0:M, so:so + F], ps[:, :], em_src, AluOpType.mult,
                    )
                    if s in ckpt_set:
                        # Renorm: scale = 1/z_row(slot s), broadcast to all
                        # partitions via outer-product matmul, folded into
                        # the emission tile of step s+2 (1 step of slack
                        # so the PE bcast hides behind evac(s+1)).
                        rc32 = rcp.tile([NC, F], dt.float32, tag="rc32")
                        rzb = rcp.tile([NC, F], dt.bfloat16, tag="rzb")
                        nc.vector.reciprocal(rc32[:, :], ring[K:M, so:so + F])
                        nc.vector.tensor_copy(rzb[:, :], rc32[:, :])
                    elif s % KREN == 1 and (s - 1) in ckpt_set:
                        bc = bcp.tile([M, F], dt.float32, tag="bc")
                        nc.tensor.matmul(bc[:, :], ind[:, :], rzb[:, :],
                                         start=True, stop=True)
                        need_ldw = True
                        emq = emqp.tile([M, F], dt.bfloat16, tag="emq")
                        nc.vector.tensor_tensor(
                            emq[:, :], bc[:, :], et[:, (ss + 1) * F:(ss + 2) * F],
                            AluOpType.mult,
                        )
                    if s % RING == RING - 1:
                        blk = s // RING
                        nc.sync.dma_start(out=zout[blk], in_=ring[K:M, :])
            nc.sync.dma_start(out=alast[:, :], in_=ring[0:K, slot(S - 1):slot(S - 1) + F])

    if strip:
        _strip_redundant_dma_waits(nc, mybir)
    return nc


def _strip_redundant_dma_waits(nc, mybir):
    """The walrus build here accepts one sem wait per instruction. On
    DMACopy the extra waits are DMAHW-queue sems for DMA-vs-DMA WAW/WAR
    on buffers that are also serialized through the engine-sem chain
    (slot readers complete before the old DMA's region is rewritten), so
    they are transitively redundant: drop them, keep engine sems.
    """
    import copy
    import json

    j = json.loads(nc.to_json_str())
    uid = [0]

    for fn in j["functions"]:
        for blk in fn["blocks"]:
            out = []
            for inst in blk["instructions"]:
                si = inst.get("sync_info")
                ow = (si.get("on_wait") or []) if si else []
                if si and len(ow) > 1 and inst.get("opcode") == "DMACopy":
                    ow = [w for w in ow if not w["ant_name"].startswith("DMAHW")]
                    assert len(ow) <= 1, f"{inst['name']}: {ow}"
                    si["on_wait"] = ow
                if si and len(ow) > 1:
                    # split a multi-wait instruction (tail Drain, or an
                    # engine op that picked up a same-engine WAR sem)
                    # into single-wait no-op Drains ahead of it
                    for w in ow[:-1]:
                        uid[0] += 1
                        out.append({
                            "name": f"{inst['name']}-w{uid[0]}",
                            "opcode": "Drain",
                            "engine": inst["engine"],
                            "debug": inst.get("debug", 0),
                            "ins": [],
                            "outs": [],
                            "is_reset_sema": False,
                            "sync_info": {"on_wait": [copy.deepcopy(w)],
                                          "on_update": []},
                        })
                    si["on_wait"] = [ow[-1]]
                out.append(inst)
            blk["instructions"] = out

    nc.m = mybir.module_from_json_bytes(json.dumps(j).encode())


def _host_score(emissions, tags, mask, transitions, start_trans, end_trans, lengths):
    b_idx = np.arange(B)
    em_tag = np.take_along_axis(emissions, tags[:, :, None], axis=2)[:, :, 0]
    trans_g = transitions[tags[:, :-1], tags[:, 1:]]
    return (start_trans[tags[:, 0]].astype(np.float64)
            + em_tag[:, 0]
            + ((trans_g + em_tag[:, 1:]) * mask[:, 1:]).sum(1, dtype=np.float64)
            + end_trans[tags[b_idx, lengths - 1]])


def kernel(emissions, tags, mask, transitions, start_trans, end_trans):
    import ml_dtypes
    from concourse.bass_utils import run_bass_kernel_spmd

    bf16 = ml_dtypes.bfloat16
    emissions = np.ascontiguousarray(np.asarray(emissions, np.float32))
    transitions = np.asarray(transitions, np.float32)
    start_trans = np.asarray(start_trans, np.float32)
    end_trans = np.asarray(end_trans, np.float32)
    tags = np.asarray(tags).astype(np.int64)
    mask = np.asarray(mask).astype(bool)

    lengths = mask.sum(1).astype(np.int64)
    score = _host_score(emissions, tags, mask, transitions, start_trans, end_trans, lengths)

    E = np.exp(transitions)
    eend = np.exp(end_trans.astype(np.float64))
    gE = float(np.log(np.exp(transitions.astype(np.float64)).sum(0)).mean())

    lhsT = np.zeros((K, M), np.float32)
    for c in range(NC):
        lhsT[T * c:T * (c + 1), T * c:T * (c + 1)] = E
        lhsT[T * c:T * (c + 1), K + c] = eend
    lhsT = lhsT.astype(bf16)
    indT = np.zeros((NC, M), np.float32)
    for c in range(NC):
        indT[c, T * c:T * (c + 1)] = 1.0
        indT[c, K + c] = 1.0
    indT = indT.astype(bf16)

    in_maps = []
    host = []  # per-core (cumg, z0)
    Ef = np.exp(transitions.astype(np.float64))
    for core in range(NCORES):
        em = emissions[core * BL:(core + 1) * BL]
        em_p = np.zeros((BLP, S, T), np.float32)
        em_p[:BL] = em
        em_p[BL:] = em[0]  # pad columns track a real row (bounded drift)
        g = em[:, :, :].max(2).mean(0).astype(np.float64) + gE
        g[0] = (em[:, 0] + start_trans[None, :]).max(1).mean(0)

        # Calibrate g so the mean per-step log-growth of the hat state is
        # ~0 (a raw mean-max+gE estimate overshoots by ~1.6/step, which
        # would underflow the bf16 ring within a renorm window). A tiny
        # f64 pre-pass on 8 sample rows measures the drift exactly.
        samp = np.linspace(0, BL - 1, 8).astype(int)
        esf = em[samp].astype(np.float64)                    # [8, S, T]
        a = np.exp(start_trans.astype(np.float64)[None, :] + esf[:, 0] - g[0])
        q = a.sum(1)
        a /= q[:, None]
        for s in range(1, S):
            a = (a @ Ef) * np.exp(esf[:, s] - g[s])
            q = a.sum(1)
            g[s] += np.log(q).mean()
            a /= q[:, None]
        cumg = np.cumsum(g)

        emt = np.exp(em_p - g[None, :, None].astype(np.float32))
        emt_dev = np.ones((S, M, F), np.float32)
        emt_dev[:, :K, :] = (
            emt.reshape(NC, F, S, T).transpose(2, 0, 3, 1).reshape(S, K, F))
        emt_dev = np.ascontiguousarray(
            emt_dev.reshape(S // KREN, KREN, M, F).transpose(0, 2, 1, 3)
            .reshape(S // KREN, M, KREN * F)).astype(bf16)

        A0 = np.exp(start_trans[None, :] + em_p[:, 0, :] - np.float32(g[0]))
        z0 = (A0.reshape(NC, F, T) * eend[None, None, :]).sum(2)  # [NC, F]
        A0d = np.empty((M, F), np.float32)
        A0d[:K] = A0.reshape(NC, F, T).transpose(0, 2, 1).reshape(K, F)
        A0d[K:] = z0
        A0d = A0d.astype(bf16)

        in_maps.append({"emt": emt_dev, "a0": A0d, "lhst": lhsT, "indt": indT})
        host.append((cumg, z0))

    nc = _BASS_CACHE.get("nc")
    if nc is None:
        nc = _build_bass()
        _BASS_CACHE["nc"] = nc

    global _LAST_IN_MAPS
    _LAST_IN_MAPS = in_maps
    res = run_bass_kernel_spmd(nc, in_maps, list(range(NCORES))).results

    nsc_tab = np.clip((np.arange(S) - 1) // KREN, 0, len(CKPTS))  # t -> #scales
    logZ = np.zeros(B, np.float64)
    for core in range(NCORES):
        cumg, z0 = host[core]
        # z-row of global slot u: zslots[u] = z_{u-1} (hat), [S, NC, F]
        zo = np.asarray(res[core]["zout"], np.float64).reshape(NBLK, NC, RING, F)
        zslots = zo.transpose(0, 2, 1, 3).reshape(S, NC, F)
        al = np.asarray(res[core]["alast"], np.float64)      # [K, F]
        zlast = (al.reshape(NC, T, F) * eend[None, :, None]).sum(1)  # [NC, F]

        # scale terms: log z-row at slots 32k (k=1..15), cumulative in k
        lsc = np.log(zslots[np.array(CKPTS), :, :])          # [15, NC, F]
        lsc_cum = np.concatenate(
            [np.zeros((1, NC, F)), np.cumsum(lsc, axis=0)], axis=0)  # [16, NC, F]

        r = np.arange(BL)
        c, n = r // F, r % F
        t = lengths[core * BL:(core + 1) * BL] - 1
        nsc = nsc_tab[t]
        zv = np.where(t == 0, z0[c, n],
                      np.where(t == S - 1, zlast[c, n],
                               zslots[np.minimum(t + 1, S - 1), c, n]))
        logZ[core * BL:(core + 1) * BL] = (
            np.log(zv) + cumg[t] + lsc_cum[nsc, c, n])
    loss = (logZ - score).mean()
    return np.array(loss, dtype=np.float32)


# revision 10
# speedup vs baseline: 1.8424x; 1.1350x over previous
"""CRF loss kernel for Trainium2 (8 NeuronCores, data-parallel over batch).

Strategy
--------
Batch 2048 is sharded 8 ways (256 rows/core). The partition function
(forward algorithm over S=512 steps) runs on device in exp-space:

  A_s[j,b] = (sum_i A_{s-1}[i,b] * E[i,j]) * exp(em[b,s,j] - g_s)

with E = exp(transitions). Per-core layout packs the 256 batch rows
(padded to 258 = 3*86) as 3 chunks on partitions: A[32c+i, n] holds batch
row c*86+n, tag i -> a [96, 86] tile. One bf16 matmul per step with
weights lhsT [96, 99] = blockdiag(E,E,E) | per-chunk exp(end_trans)
columns, so psum rows 96:99 of step s give z_{s-1} = sum_i A_{s-1} * eend
(the masked-length readout) for free. The weights are constant, so all
matmuls after the first run with ldweights=False (PE array reuse). One
DVE multiply per step applies the emission factor and stores the step's
A+z into a 64-slot bf16 SBUF ring; z rows are DMA'd out per 64-step block.

Stability: a host-computed per-step scalar g_s keeps A near 1; every 32
steps the z-row of the current slot (a natural per-column magnitude
proxy) is reciprocal'd, broadcast across partitions by a tiny
outer-product matmul (indT weights at PE rows 96:99 would clobber the
main weights' rows 0:32, so the next main matmul reloads), and folded
into the emission tile two steps ahead. The host reconstructs
logZ_b = log(z_row[L_b]) + cumg + sum of log(z_row[32k]) scale terms,
all read from zout; the gold-path score is computed in numpy.
"""
import os
import sys

import numpy as np

for _p in ("/opt/trn_rl_repo",):
    if _p not in sys.path and os.path.isdir(_p):
        sys.path.insert(0, _p)

B, S, T = 2048, 512, 32
NCORES = 8
BL = B // NCORES          # 256 batch rows per core
F = 86                    # batch columns per chunk
NC = 3                    # chunks per core
BLP = NC * F              # 258 padded rows
K = NC * T                # 96 contraction partitions
M = K + NC                # 99 matmul output rows (incl. z rows)
KREN = 32                 # em DMA block granularity
CKPT = 64                 # renorm period
RING = 64                 # ring slots
NBLK = S // RING          # 8 z export blocks
CKPTS = list(range(CKPT, S, CKPT))          # 64..448, applied at p+2
H = 2                     # interleaved chains (column halves)
F2 = F // H               # 43 columns per chain

_BASS_CACHE = {}


def _build_bass(strip=True):
    import concourse.bass as bass
    import concourse.mybir as mybir
    from concourse.alu_op_type import AluOpType
    from concourse.tile import TileContext

    dt = mybir.dt
    nc = bass.Bass()

    emt = nc.declare_dram_parameter("emt", [S // KREN, M, KREN * F], dt.bfloat16, isOutput=False)
    a0 = nc.declare_dram_parameter("a0", [M, F], dt.bfloat16, isOutput=False)
    lhst = nc.declare_dram_parameter("lhst", [K, M], dt.bfloat16, isOutput=False)
    indt = nc.declare_dram_parameter("indt", [NC, M], dt.bfloat16, isOutput=False)
    zout = nc.declare_dram_parameter("zout", [NBLK, NC, RING * F], dt.bfloat16, isOutput=True)
    alast = nc.declare_dram_parameter("alast", [K, F], dt.bfloat16, isOutput=True)

    ckpt_set = set(CKPTS)

    with TileContext(nc) as tc:
        with (
            tc.tile_pool(name="const", bufs=1) as constp,
            tc.tile_pool(name="ringp", bufs=1) as ringp,
            tc.tile_pool(name="emts", bufs=3) as emtp,
            tc.tile_pool(name="rcp", bufs=2) as rcp,
            tc.tile_pool(name="emqp", bufs=2) as emqp,
            tc.tile_pool(name="mmpsum", bufs=2, space="PSUM") as mmp,
            tc.tile_pool(name="bcpsum", bufs=2, space="PSUM") as bcp,
        ):
            lh = constp.tile([K, M], dt.bfloat16, name="lh")
            ind = constp.tile([NC, M], dt.bfloat16, name="ind")
            ring = ringp.tile([M, RING * F], dt.bfloat16, name="ring")

            # Stage matmul-consumed constants through DVE copies so every
            # matmul dependency is a single DVE semaphore (the fused
            # LDWEIGHTS+MATMUL struct has a tiny sync-wait budget).
            lhs_s = constp.tile([K, M], dt.bfloat16, name="lhs_s")
            ind_s = constp.tile([NC, M], dt.bfloat16, name="ind_s")
            a0_s = constp.tile([M, F], dt.bfloat16, name="a0_s")
            nc.sync.dma_start(out=lhs_s[:, :], in_=lhst[:, :])
            nc.sync.dma_start(out=ind_s[:, :], in_=indt[:, :])
            nc.sync.dma_start(out=a0_s[:, :], in_=a0[:, :])
            nc.vector.tensor_copy(lh[:, :], lhs_s[:, :])
            nc.vector.tensor_copy(ind[:, :], ind_s[:, :])
            nc.vector.tensor_copy(ring[0:M, 0:F], a0_s[:, :])
            dum = constp.tile([1, S // KREN], dt.float32, name="dum")

            def slot(u):
                return (u % RING) * F

            emq = None          # renorm-scaled emission tile for step p+2
            need_ldw = True     # PE weight rows 0:96 need (re)loading
            for s16 in range(S // KREN):
                et = emtp.tile([M, KREN * F], dt.bfloat16, tag="et")
                nc.sync.dma_start(out=et[:, :], in_=emt[s16])
                # Sponges: each engine instruction struct holds a single
                # sem wait, so route DMA waits through cheap DVE ops with
                # no data-dep back into the scan (ordering via add_dep).
                nc.vector.tensor_copy(dum[0:1, s16:s16 + 1], et[0:1, 0:1])
                if s16 % (RING // KREN) == 0 and s16 > 0:
                    # WAR sponge: zout DMA of the previous 64-block must
                    # finish before this block's evacs rewrite z rows.
                    nc.vector.memset(ring[K:K + 1, 0:1], 0.0)
                for ss in range(KREN):
                    s = s16 * KREN + ss
                    if s == 0:
                        continue
                    po = slot(s - 1)
                    so = slot(s)
                    # Two interleaved chains (column halves): chain B's
                    # matmul/evac overlap chain A's, hiding the per-step
                    # round-trip latency. All main matmuls reuse the PE
                    # array weights (ldweights=True -> no reload) except
                    # after a ckpt bcast clobbers rows 0:32.
                    pss = []
                    for h in range(H):
                        ps = mmp.tile([M, F2], dt.float32, tag=f"ps{h}")
                        mm = nc.tensor.matmul(
                            ps[:, :], lh[:, :],
                            ring[0:K, po + F2 * h:po + F2 * (h + 1)],
                            start=True, stop=True)
                        if need_ldw:
                            need_ldw = False
                        else:
                            mm.ins.ldweights = True
                        pss.append(ps)
                    if s % CKPT == 2 and (s - 2) in ckpt_set:
                        em_src = emq
                        em_off = 0
                    else:
                        em_src = et
                        em_off = ss * F
                    for h in range(H):
                        nc.vector.tensor_tensor(
                            ring[0:M, so + F2 * h:so + F2 * (h + 1)],
                            pss[h][:, :],
                            em_src[:, em_off + F2 * h:em_off + F2 * (h + 1)],
                            AluOpType.mult,
                        )
                    if s in ckpt_set:
                        # Renorm: scale = 1/z_row(slot s), broadcast to all
                        # partitions via outer-product matmul, folded into
                        # the emission tile of step s+2 (1 step of slack
                        # so the PE bcast hides behind evac(s+1)).
                        rc32 = rcp.tile([NC, F], dt.float32, tag="rc32")
                        rzb = rcp.tile([NC, F], dt.bfloat16, tag="rzb")
                        nc.vector.reciprocal(rc32[:, :], ring[K:M, so:so + F])
                        nc.vector.tensor_copy(rzb[:, :], rc32[:, :])
                    elif s % CKPT == 1 and (s - 1) in ckpt_set:
                        bc = bcp.tile([M, F], dt.float32, tag="bc")
                        nc.tensor.matmul(bc[:, :], ind[:, :], rzb[:, :],
                                         start=True, stop=True)
                        need_ldw = True
                        emq = emqp.tile([M, F], dt.bfloat16, tag="emq")
                        nc.vector.tensor_tensor(
                            emq[:, :], bc[:, :], et[:, (ss + 1) * F:(ss + 2) * F],
                            AluOpType.mult,
                        )
                    if s % RING == RING - 1:
                        blk = s // RING
                        nc.sync.dma_start(out=zout[blk], in_=ring[K:M, :])
            nc.sync.dma_start(out=alast[:, :], in_=ring[0:K, slot(S - 1):slot(S - 1) + F])

    if strip:
        _strip_redundant_dma_waits(nc, mybir)
    return nc


def _strip_redundant_dma_waits(nc, mybir):
    """The walrus build here accepts one sem wait per instruction. On
    DMACopy the extra waits are DMAHW-queue sems for DMA-vs-DMA WAW/WAR
    on buffers that are also serialized through the engine-sem chain
    (slot readers complete before the old DMA's region is rewritten), so
    they are transitively redundant: drop them, keep engine sems.
    """
    import copy
    import json

    j = json.loads(nc.to_json_str())
    uid = [0]

    for fn in j["functions"]:
        for blk in fn["blocks"]:
            out = []
            for inst in blk["instructions"]:
                si = inst.get("sync_info")
                ow = (si.get("on_wait") or []) if si else []
                if si and len(ow) > 1 and inst.get("opcode") == "DMACopy":
                    ow = [w for w in ow if not w["ant_name"].startswith("DMAHW")]
                    assert len(ow) <= 1, f"{inst['name']}: {ow}"
                    si["on_wait"] = ow
                if si and len(ow) > 1:
                    # split a multi-wait instruction (tail Drain, or an
                    # engine op that picked up a same-engine WAR sem)
                    # into single-wait no-op Drains ahead of it
                    for w in ow[:-1]:
                        uid[0] += 1
                        out.append({
                            "name": f"{inst['name']}-w{uid[0]}",
                            "opcode": "Drain",
                            "engine": inst["engine"],
                            "debug": inst.get("debug", 0),
                            "ins": [],
                            "outs": [],
                            "is_reset_sema": False,
                            "sync_info": {"on_wait": [copy.deepcopy(w)],
                                          "on_update": []},
                        })
                    si["on_wait"] = [ow[-1]]
                out.append(inst)
            blk["instructions"] = out

    nc.m = mybir.module_from_json_bytes(json.dumps(j).encode())


def _host_score(emissions, tags, mask, transitions, start_trans, end_trans, lengths):
    b_idx = np.arange(B)
    em_tag = np.take_along_axis(emissions, tags[:, :, None], axis=2)[:, :, 0]
    trans_g = transitions[tags[:, :-1], tags[:, 1:]]
    return (start_trans[tags[:, 0]].astype(np.float64)
            + em_tag[:, 0]
            + ((trans_g + em_tag[:, 1:]) * mask[:, 1:]).sum(1, dtype=np.float64)
            + end_trans[tags[b_idx, lengths - 1]])


def kernel(emissions, tags, mask, transitions, start_trans, end_trans):
    import ml_dtypes
    from concourse.bass_utils import run_bass_kernel_spmd

    bf16 = ml_dtypes.bfloat16
    emissions = np.ascontiguousarray(np.asarray(emissions, np.float32))
    transitions = np.asarray(transitions, np.float32)
    start_trans = np.asarray(start_trans, np.float32)
    end_trans = np.asarray(end_trans, np.float32)
    tags = np.asarray(tags).astype(np.int64)
    mask = np.asarray(mask).astype(bool)

    lengths = mask.sum(1).astype(np.int64)
    score = _host_score(emissions, tags, mask, transitions, start_trans, end_trans, lengths)

    E = np.exp(transitions)
    eend = np.exp(end_trans.astype(np.float64))
    gE = float(np.log(np.exp(transitions.astype(np.float64)).sum(0)).mean())

    lhsT = np.zeros((K, M), np.float32)
    for c in range(NC):
        lhsT[T * c:T * (c + 1), T * c:T * (c + 1)] = E
        lhsT[T * c:T * (c + 1), K + c] = eend
    lhsT = lhsT.astype(bf16)
    indT = np.zeros((NC, M), np.float32)
    for c in range(NC):
        indT[c, T * c:T * (c + 1)] = 1.0
        indT[c, K + c] = 1.0
    indT = indT.astype(bf16)

    in_maps = []
    host = []  # per-core (cumg, z0)
    Ef = np.exp(transitions.astype(np.float64))
    for core in range(NCORES):
        em = emissions[core * BL:(core + 1) * BL]
        em_p = np.zeros((BLP, S, T), np.float32)
        em_p[:BL] = em
        em_p[BL:] = em[0]  # pad columns track a real row (bounded drift)
        g = em[:, :, :].max(2).mean(0).astype(np.float64) + gE
        g[0] = (em[:, 0] + start_trans[None, :]).max(1).mean(0)

        # Calibrate g so the mean per-step log-growth of the hat state is
        # ~0 (a raw mean-max+gE estimate overshoots by ~1.6/step, which
        # would underflow the bf16 ring within a renorm window). A tiny
        # f64 pre-pass on 8 sample rows measures the drift exactly.
        samp = np.linspace(0, BL - 1, 8).astype(int)
        esf = em[samp].astype(np.float64)                    # [8, S, T]
        a = np.exp(start_trans.astype(np.float64)[None, :] + esf[:, 0] - g[0])
        q = a.sum(1)
        a /= q[:, None]
        for s in range(1, S):
            a = (a @ Ef) * np.exp(esf[:, s] - g[s])
            q = a.sum(1)
            g[s] += np.log(q).mean()
            a /= q[:, None]
        cumg = np.cumsum(g)

        emt = np.exp(em_p - g[None, :, None].astype(np.float32))
        emt_dev = np.ones((S, M, F), np.float32)
        emt_dev[:, :K, :] = (
            emt.reshape(NC, F, S, T).transpose(2, 0, 3, 1).reshape(S, K, F))
        emt_dev = np.ascontiguousarray(
            emt_dev.reshape(S // KREN, KREN, M, F).transpose(0, 2, 1, 3)
            .reshape(S // KREN, M, KREN * F)).astype(bf16)

        A0 = np.exp(start_trans[None, :] + em_p[:, 0, :] - np.float32(g[0]))
        z0 = (A0.reshape(NC, F, T) * eend[None, None, :]).sum(2)  # [NC, F]
        A0d = np.empty((M, F), np.float32)
        A0d[:K] = A0.reshape(NC, F, T).transpose(0, 2, 1).reshape(K, F)
        A0d[K:] = z0
        A0d = A0d.astype(bf16)

        in_maps.append({"emt": emt_dev, "a0": A0d, "lhst": lhsT, "indt": indT})
        host.append((cumg, z0))

    nc = _BASS_CACHE.get("nc")
    if nc is None:
        nc = _build_bass()
        _BASS_CACHE["nc"] = nc

    global _LAST_IN_MAPS
    _LAST_IN_MAPS = in_maps
    res = run_bass_kernel_spmd(nc, in_maps, list(range(NCORES))).results

    nsc_tab = np.clip((np.arange(S) - 1) // CKPT, 0, len(CKPTS))  # t -> #scales
    logZ = np.zeros(B, np.float64)
    for core in range(NCORES):
        cumg, z0 = host[core]
        # z-row of global slot u: zslots[u] = z_{u-1} (hat), [S, NC, F]
        zo = np.asarray(res[core]["zout"], np.float64).reshape(NBLK, NC, RING, F)
        zslots = zo.transpose(0, 2, 1, 3).reshape(S, NC, F)
        al = np.asarray(res[core]["alast"], np.float64)      # [K, F]
        zlast = (al.reshape(NC, T, F) * eend[None, :, None]).sum(1)  # [NC, F]

        # scale terms: log z-row at slots 32k (k=1..15), cumulative in k
        lsc = np.log(zslots[np.array(CKPTS), :, :])          # [15, NC, F]
        lsc_cum = np.concatenate(
            [np.zeros((1, NC, F)), np.cumsum(lsc, axis=0)], axis=0)  # [16, NC, F]

        r = np.arange(BL)
        c, n = r // F, r % F
        t = lengths[core * BL:(core + 1) * BL] - 1
        nsc = nsc_tab[t]
        zv = np.where(t == 0, z0[c, n],
                      np.where(t == S - 1, zlast[c, n],
                               zslots[np.minimum(t + 1, S - 1), c, n]))
        logZ[core * BL:(core + 1) * BL] = (
            np.log(zv) + cumg[t] + lsc_cum[nsc, c, n])
    loss = (logZ - score).mean()
    return np.array(loss, dtype=np.float32)


# revision 11
# speedup vs baseline: 1.8483x; 1.0032x over previous
"""CRF loss kernel for Trainium2 (8 NeuronCores, data-parallel over batch).

Strategy
--------
Batch 2048 is sharded 8 ways (256 rows/core). The partition function
(forward algorithm over S=512 steps) runs on device in exp-space:

  A_s[j,b] = (sum_i A_{s-1}[i,b] * E[i,j]) * exp(em[b,s,j] - g_s)

with E = exp(transitions). Per-core layout packs the 256 batch rows
(padded to 258 = 3*86) as 3 chunks on partitions: A[32c+i, n] holds batch
row c*86+n, tag i -> a [96, 86] tile. One bf16 matmul per step with
weights lhsT [96, 99] = blockdiag(E,E,E) | per-chunk exp(end_trans)
columns, so psum rows 96:99 of step s give z_{s-1} = sum_i A_{s-1} * eend
(the masked-length readout) for free. The weights are constant, so all
matmuls after the first run with ldweights=False (PE array reuse). One
DVE multiply per step applies the emission factor and stores the step's
A+z into a 64-slot bf16 SBUF ring; z rows are DMA'd out per 64-step block.

Stability: a host-computed per-step scalar g_s keeps A near 1; every 32
steps the z-row of the current slot (a natural per-column magnitude
proxy) is reciprocal'd, broadcast across partitions by a tiny
outer-product matmul (indT weights at PE rows 96:99 would clobber the
main weights' rows 0:32, so the next main matmul reloads), and folded
into the emission tile two steps ahead. The host reconstructs
logZ_b = log(z_row[L_b]) + cumg + sum of log(z_row[32k]) scale terms,
all read from zout; the gold-path score is computed in numpy.
"""
import os
import sys

import numpy as np

for _p in ("/opt/trn_rl_repo",):
    if _p not in sys.path and os.path.isdir(_p):
        sys.path.insert(0, _p)

B, S, T = 2048, 512, 32
NCORES = 8
BL = B // NCORES          # 256 batch rows per core
F = 86                    # batch columns per chunk
NC = 3                    # chunks per core
BLP = NC * F              # 258 padded rows
K = NC * T                # 96 contraction partitions
M = K + NC                # 99 matmul output rows (incl. z rows)
KREN = 32                 # em DMA block granularity
CKPT = 64                 # renorm period
RING = 64                 # ring slots
NBLK = S // RING          # 8 z export blocks
CKPTS = list(range(CKPT, S, CKPT))          # 64..448, applied at p+2
H = 2                     # interleaved chains (column halves)
F2 = F // H               # 43 columns per chain

_BASS_CACHE = {}


def _build_bass(strip=True):
    import concourse.bass as bass
    import concourse.mybir as mybir
    from concourse.alu_op_type import AluOpType
    from concourse.tile import TileContext

    dt = mybir.dt
    nc = bass.Bass()

    emt = nc.declare_dram_parameter("emt", [S // KREN, M, KREN * F], dt.bfloat16, isOutput=False)
    a0 = nc.declare_dram_parameter("a0", [M, F], dt.bfloat16, isOutput=False)
    lhst = nc.declare_dram_parameter("lhst", [K, M], dt.bfloat16, isOutput=False)
    indt = nc.declare_dram_parameter("indt", [NC, M], dt.bfloat16, isOutput=False)
    zout = nc.declare_dram_parameter("zout", [NBLK, NC, RING * F], dt.bfloat16, isOutput=True)
    alast = nc.declare_dram_parameter("alast", [K, F], dt.bfloat16, isOutput=True)

    ckpt_set = set(CKPTS)

    with TileContext(nc) as tc:
        with (
            tc.tile_pool(name="const", bufs=1) as constp,
            tc.tile_pool(name="ringp", bufs=1) as ringp,
            tc.tile_pool(name="emts", bufs=3) as emtp,
            tc.tile_pool(name="rcp", bufs=2) as rcp,
            tc.tile_pool(name="emqp", bufs=2) as emqp,
            tc.tile_pool(name="mmpsum", bufs=2, space="PSUM") as mmp,
            tc.tile_pool(name="bcpsum", bufs=2, space="PSUM") as bcp,
        ):
            lh = constp.tile([K, M], dt.bfloat16, name="lh")
            ind = constp.tile([NC, M], dt.bfloat16, name="ind")
            ring = ringp.tile([M, RING * F], dt.bfloat16, name="ring")

            # Stage matmul-consumed constants through DVE copies so every
            # matmul dependency is a single DVE semaphore (the fused
            # LDWEIGHTS+MATMUL struct has a tiny sync-wait budget).
            lhs_s = constp.tile([K, M], dt.bfloat16, name="lhs_s")
            ind_s = constp.tile([NC, M], dt.bfloat16, name="ind_s")
            a0_s = constp.tile([M, F], dt.bfloat16, name="a0_s")
            nc.sync.dma_start(out=lhs_s[:, :], in_=lhst[:, :])
            nc.sync.dma_start(out=ind_s[:, :], in_=indt[:, :])
            nc.sync.dma_start(out=a0_s[:, :], in_=a0[:, :])
            nc.vector.tensor_copy(lh[:, :], lhs_s[:, :])
            nc.vector.tensor_copy(ind[:, :], ind_s[:, :])
            nc.vector.tensor_copy(ring[0:M, 0:F], a0_s[:, :])
            dum = constp.tile([1, S // KREN], dt.float32, name="dum")

            def slot(u):
                return (u % RING) * F

            emq = None          # renorm-scaled emission tile for step p+2
            need_ldw = True     # PE weight rows 0:96 need (re)loading
            for s16 in range(S // KREN):
                et = emtp.tile([M, KREN * F], dt.bfloat16, tag="et")
                nc.sync.dma_start(out=et[:, :], in_=emt[s16])
                # Sponges: each engine instruction struct holds a single
                # sem wait, so route DMA waits through cheap DVE ops with
                # no data-dep back into the scan (ordering via add_dep).
                nc.vector.tensor_copy(dum[0:1, s16:s16 + 1], et[0:1, 0:1])
                if s16 % (RING // KREN) == 0 and s16 > 0:
                    # WAR sponge: zout DMA of the previous 64-block must
                    # finish before this block's evacs rewrite z rows.
                    nc.vector.memset(ring[K:K + 1, 0:1], 0.0)
                for ss in range(KREN):
                    s = s16 * KREN + ss
                    if s == 0:
                        continue
                    po = slot(s - 1)
                    so = slot(s)
                    # Two interleaved chains (column halves): chain B's
                    # matmul/evac overlap chain A's, hiding the per-step
                    # round-trip latency. All main matmuls reuse the PE
                    # array weights (ldweights=True -> no reload) except
                    # after a ckpt bcast clobbers rows 0:32.
                    pss = []
                    for h in range(H):
                        ps = mmp.tile([M, F2], dt.float32, tag=f"ps{h}")
                        if need_ldw:
                            nc.tensor.ldweights(lh[:, :])
                            need_ldw = False
                        mm = nc.tensor.matmul(
                            ps[:, :], lh[:, :],
                            ring[0:K, po + F2 * h:po + F2 * (h + 1)],
                            start=True, stop=True)
                        mm.ins.ldweights = True
                        pss.append(ps)
                    if s % CKPT == 2 and (s - 2) in ckpt_set:
                        em_src = emq
                        em_off = 0
                    else:
                        em_src = et
                        em_off = ss * F
                    for h in range(H):
                        nc.vector.tensor_tensor(
                            ring[0:M, so + F2 * h:so + F2 * (h + 1)],
                            pss[h][:, :],
                            em_src[:, em_off + F2 * h:em_off + F2 * (h + 1)],
                            AluOpType.mult,
                        )
                    if s in ckpt_set:
                        # Renorm: scale = 1/z_row(slot s), broadcast to all
                        # partitions via outer-product matmul, folded into
                        # the emission tile of step s+2 (1 step of slack
                        # so the PE bcast hides behind evac(s+1)).
                        rc32 = rcp.tile([NC, F], dt.float32, tag="rc32")
                        rzb = rcp.tile([NC, F], dt.bfloat16, tag="rzb")
                        nc.vector.reciprocal(rc32[:, :], ring[K:M, so:so + F])
                        nc.vector.tensor_copy(rzb[:, :], rc32[:, :])
                    elif s % CKPT == 1 and (s - 1) in ckpt_set:
                        bc = bcp.tile([M, F], dt.float32, tag="bc")
                        nc.tensor.matmul(bc[:, :], ind[:, :], rzb[:, :],
                                         start=True, stop=True)
                        need_ldw = True
                        emq = emqp.tile([M, F], dt.bfloat16, tag="emq")
                        nc.vector.tensor_tensor(
                            emq[:, :], bc[:, :], et[:, (ss + 1) * F:(ss + 2) * F],
                            AluOpType.mult,
                        )
                    if s % RING == RING - 1:
                        blk = s // RING
                        nc.sync.dma_start(out=zout[blk], in_=ring[K:M, :])
            nc.sync.dma_start(out=alast[:, :], in_=ring[0:K, slot(S - 1):slot(S - 1) + F])

    if strip:
        _strip_redundant_dma_waits(nc, mybir)
    return nc


def _strip_redundant_dma_waits(nc, mybir):
    """The walrus build here accepts one sem wait per instruction. On
    DMACopy the extra waits are DMAHW-queue sems for DMA-vs-DMA WAW/WAR
    on buffers that are also serialized through the engine-sem chain
    (slot readers complete before the old DMA's region is rewritten), so
    they are transitively redundant: drop them, keep engine sems.
    """
    import copy
    import json

    j = json.loads(nc.to_json_str())
    uid = [0]

    for fn in j["functions"]:
        for blk in fn["blocks"]:
            out = []
            for inst in blk["instructions"]:
                si = inst.get("sync_info")
                ow = (si.get("on_wait") or []) if si else []
                if si and len(ow) > 1 and inst.get("opcode") == "DMACopy":
                    ow = [w for w in ow if not w["ant_name"].startswith("DMAHW")]
                    assert len(ow) <= 1, f"{inst['name']}: {ow}"
                    si["on_wait"] = ow
                if si and len(ow) > 1:
                    # split a multi-wait instruction (tail Drain, or an
                    # engine op that picked up a same-engine WAR sem)
                    # into single-wait no-op Drains ahead of it
                    for w in ow[:-1]:
                        uid[0] += 1
                        out.append({
                            "name": f"{inst['name']}-w{uid[0]}",
                            "opcode": "Drain",
                            "engine": inst["engine"],
                            "debug": inst.get("debug", 0),
                            "ins": [],
                            "outs": [],
                            "is_reset_sema": False,
                            "sync_info": {"on_wait": [copy.deepcopy(w)],
                                          "on_update": []},
                        })
                    si["on_wait"] = [ow[-1]]
                out.append(inst)
            blk["instructions"] = out

    nc.m = mybir.module_from_json_bytes(json.dumps(j).encode())


def _host_score(emissions, tags, mask, transitions, start_trans, end_trans, lengths):
    b_idx = np.arange(B)
    em_tag = np.take_along_axis(emissions, tags[:, :, None], axis=2)[:, :, 0]
    trans_g = transitions[tags[:, :-1], tags[:, 1:]]
    return (start_trans[tags[:, 0]].astype(np.float64)
            + em_tag[:, 0]
            + ((trans_g + em_tag[:, 1:]) * mask[:, 1:]).sum(1, dtype=np.float64)
            + end_trans[tags[b_idx, lengths - 1]])


def kernel(emissions, tags, mask, transitions, start_trans, end_trans):
    import ml_dtypes
    from concourse.bass_utils import run_bass_kernel_spmd

    bf16 = ml_dtypes.bfloat16
    emissions = np.ascontiguousarray(np.asarray(emissions, np.float32))
    transitions = np.asarray(transitions, np.float32)
    start_trans = np.asarray(start_trans, np.float32)
    end_trans = np.asarray(end_trans, np.float32)
    tags = np.asarray(tags).astype(np.int64)
    mask = np.asarray(mask).astype(bool)

    lengths = mask.sum(1).astype(np.int64)
    score = _host_score(emissions, tags, mask, transitions, start_trans, end_trans, lengths)

    E = np.exp(transitions)
    eend = np.exp(end_trans.astype(np.float64))
    gE = float(np.log(np.exp(transitions.astype(np.float64)).sum(0)).mean())

    lhsT = np.zeros((K, M), np.float32)
    for c in range(NC):
        lhsT[T * c:T * (c + 1), T * c:T * (c + 1)] = E
        lhsT[T * c:T * (c + 1), K + c] = eend
    lhsT = lhsT.astype(bf16)
    indT = np.zeros((NC, M), np.float32)
    for c in range(NC):
        indT[c, T * c:T * (c + 1)] = 1.0
        indT[c, K + c] = 1.0
    indT = indT.astype(bf16)

    in_maps = []
    host = []  # per-core (cumg, z0)
    Ef = np.exp(transitions.astype(np.float64))
    for core in range(NCORES):
        em = emissions[core * BL:(core + 1) * BL]
        em_p = np.zeros((BLP, S, T), np.float32)
        em_p[:BL] = em
        em_p[BL:] = em[0]  # pad columns track a real row (bounded drift)
        g = em[:, :, :].max(2).mean(0).astype(np.float64) + gE
        g[0] = (em[:, 0] + start_trans[None, :]).max(1).mean(0)

        # Calibrate g so the mean per-step log-growth of the hat state is
        # ~0 (a raw mean-max+gE estimate overshoots by ~1.6/step, which
        # would underflow the bf16 ring within a renorm window). A tiny
        # f64 pre-pass on 8 sample rows measures the drift exactly.
        samp = np.linspace(0, BL - 1, 8).astype(int)
        esf = em[samp].astype(np.float64)                    # [8, S, T]
        a = np.exp(start_trans.astype(np.float64)[None, :] + esf[:, 0] - g[0])
        q = a.sum(1)
        a /= q[:, None]
        for s in range(1, S):
            a = (a @ Ef) * np.exp(esf[:, s] - g[s])
            q = a.sum(1)
            g[s] += np.log(q).mean()
            a /= q[:, None]
        cumg = np.cumsum(g)

        emt = np.exp(em_p - g[None, :, None].astype(np.float32))
        emt_dev = np.ones((S, M, F), np.float32)
        emt_dev[:, :K, :] = (
            emt.reshape(NC, F, S, T).transpose(2, 0, 3, 1).reshape(S, K, F))
        emt_dev = np.ascontiguousarray(
            emt_dev.reshape(S // KREN, KREN, M, F).transpose(0, 2, 1, 3)
            .reshape(S // KREN, M, KREN * F)).astype(bf16)

        A0 = np.exp(start_trans[None, :] + em_p[:, 0, :] - np.float32(g[0]))
        z0 = (A0.reshape(NC, F, T) * eend[None, None, :]).sum(2)  # [NC, F]
        A0d = np.empty((M, F), np.float32)
        A0d[:K] = A0.reshape(NC, F, T).transpose(0, 2, 1).reshape(K, F)
        A0d[K:] = z0
        A0d = A0d.astype(bf16)

        in_maps.append({"emt": emt_dev, "a0": A0d, "lhst": lhsT, "indt": indT})
        host.append((cumg, z0))

    nc = _BASS_CACHE.get("nc")
    if nc is None:
        nc = _build_bass()
        _BASS_CACHE["nc"] = nc

    global _LAST_IN_MAPS
    _LAST_IN_MAPS = in_maps
    res = run_bass_kernel_spmd(nc, in_maps, list(range(NCORES))).results

    nsc_tab = np.clip((np.arange(S) - 1) // CKPT, 0, len(CKPTS))  # t -> #scales
    logZ = np.zeros(B, np.float64)
    for core in range(NCORES):
        cumg, z0 = host[core]
        # z-row of global slot u: zslots[u] = z_{u-1} (hat), [S, NC, F]
        zo = np.asarray(res[core]["zout"], np.float64).reshape(NBLK, NC, RING, F)
        zslots = zo.transpose(0, 2, 1, 3).reshape(S, NC, F)
        al = np.asarray(res[core]["alast"], np.float64)      # [K, F]
        zlast = (al.reshape(NC, T, F) * eend[None, :, None]).sum(1)  # [NC, F]

        # scale terms: log z-row at slots 32k (k=1..15), cumulative in k
        lsc = np.log(zslots[np.array(CKPTS), :, :])          # [15, NC, F]
        lsc_cum = np.concatenate(
            [np.zeros((1, NC, F)), np.cumsum(lsc, axis=0)], axis=0)  # [16, NC, F]

        r = np.arange(BL)
        c, n = r // F, r % F
        t = lengths[core * BL:(core + 1) * BL] - 1
        nsc = nsc_tab[t]
        zv = np.where(t == 0, z0[c, n],
                      np.where(t == S - 1, zlast[c, n],
                               zslots[np.minimum(t + 1, S - 1), c, n]))
        logZ[core * BL:(core + 1) * BL] = (
            np.log(zv) + cumg[t] + lsc_cum[nsc, c, n])
    loss = (logZ - score).mean()
    return np.array(loss, dtype=np.float32)
